# revision 46
# baseline (speedup 1.0000x reference)
"""Trainium2 Bass kernel for nn_BsplineLoss (chamfer between skeletal points
and bspline curve points).

Full-input contract: kernel(**inputs) takes the unsharded arrays
  skeletal_points      (16, 4096, 3) f32
  primitive_parameters (16, 64, 12)  f32
  bspline_basis        (16, 4)       f32
and returns the full (16,) f32 loss.

Sharding: data-parallel over batch B=16 across 8 cores (2 batches/core).

Device algorithm (per core, per batch), "m-on-partitions" orientation:
  curves b = einsum(basis, ctrl)                       (M=1024 points)
  psum[m, p] = 2*a_p.b_m - |b_m|^2 - |a_p|^2 = -d2     (K=13 hi/lo bf16)
  for each m-block q (128 curve points on partitions, p free):
    sbd(q) = Relu(-psum) = d2  (bf16; two (128,2048) ScalarE drains)
    colraw[r, q] = min_p sbd   (one DVE custom 2-stream min+accum, FD=2048)
    run = min(run, sbd)        (one DVE tensor_tensor min, FD=4096, 2x mode)
Host: fold the 128-partition axis of run (rowmin over the 8 m-blocks is in
run already; partitions hold m mod 128) and relu/sqrt/mean -> loss; colmin
comes from colraw directly (m = q*128 + r).

The steady state is ScalarE-drain-paced (~2 x 1.97us per m-block); the DVE
(custom min at 1x + TT at 2x) runs ~0.5us/block behind, so the final batch's
last three colmin customs are skipped entirely: their raw d2 tiles are dumped
to DRAM on the idle sync HW-DGE queue and min-folded on the host (osbd).
"""

import numpy as np

P = 128
NB = 2           # batches per core
PPB = 4096       # skeletal points per batch
M = 1024         # curve points per batch
MB = 8           # m-blocks per batch (128 m's each)
HP = 2048        # p-half size
JPP = 32
NCORES = 8

_CACHE = {}


def _register_min_op():
    """Custom DVE op: out = min(in0, in1); accum_out = min(c0, min_k out).
    Reads two SBUF bf16 streams at 2 elem/cycle/lane total (2x perf mode)."""
    from concourse import dve_ops
    from concourse.dve_spec import Spec, minn, Src0, Src1, C0, lower, _has_src1
    from concourse.dve_uop import DveOpSpec

    name = "TT_MIN_RED_ANT"
    for o in dve_ops.OPS:
        if o.name == name:
            return o

    def _ref(in0, in1, c0, c1, c2):
        body = np.minimum(in0.astype(np.float32), in1.astype(np.float32))
        acc = np.minimum(
            c0, body.reshape(body.shape[0], -1).min(axis=-1, keepdims=True)
        )
        return body, acc

    spec = Spec(body=minn(Src0, Src1), accum=minn, accum_init=C0, reference=_ref)
    opcode = max(dve_ops._SUB_OPCODE_FOR_NAME.values()) + 1
    assert opcode < 0x20
    shas = {}
    for ver in ("v3", "v4"):
        try:
            s = DveOpSpec(
                name=name, opcode=opcode, uops=lower(spec, ver=ver),
                rd1_en=_has_src1(spec),
            )
            shas[ver] = s.sha(ver)
        except Exception:
            pass
    op = dve_ops.DveOp(name, spec, subdim=False, uops_sha=shas,
                       perf_en={"v3": True, "v4": True})
    dve_ops.OPS.append(op)
    dve_ops.CUSTOM_DVE_SPECS[name] = spec
    dve_ops._SUB_OPCODE_FOR_NAME[name] = opcode
    return op


def _build_nc():
    import concourse.bacc as bacc
    import concourse.bass as bass
    import concourse.tile as tile
    from concourse import mybir, bass_isa

    f32 = mybir.dt.float32
    bf16 = mybir.dt.bfloat16
    AX = mybir.AxisListType
    AL = mybir.AluOpType
    ACT = mybir.ActivationFunctionType

    min_op = _register_min_op()
    nc = bacc.Bacc(None, target_bir_lowering=False)

    skel = nc.dram_tensor("skel", [NB * PPB, 3], f32, kind="ExternalInput")
    prim = nc.dram_tensor("prim", [P, 12], f32, kind="ExternalInput")
    basis = nc.dram_tensor("basis", [16, 4], f32, kind="ExternalInput")

    # rowmin side: running min over m-blocks, host folds the 128 partitions
    orun = nc.dram_tensor("orun", [NB, P, PPB], bf16, kind="ExternalOutput")
    # colmin side: one free-axis min column per m-block (m = q*128 + r)
    ocolr = nc.dram_tensor("ocolr", [P, NB * MB], f32, kind="ExternalOutput")
    # raw d2 tiles for the final batch's last 3 m-blocks (host folds colmin);
    # dumped via the idle sync HW-DGE queue, NOT gpsimd SWDGE (too slow)
    osbd = nc.dram_tensor("osbd", [3, P, PPB], bf16, kind="ExternalOutput")

    scratch = nc.dram_tensor("scratch", [P, 128], bf16)
    scratch_a = nc.dram_tensor("scratch_a", [NB, P, 13 * JPP], bf16)

    ident_dram = nc.inline_tensor(np.eye(P, dtype=np.float32), name="ident")

    with tile.TileContext(nc) as tc:
        with (
            tc.tile_pool(name="const", bufs=1) as constp,
            tc.tile_pool(name="prep", bufs=2) as prep,
            tc.tile_pool(name="persist", bufs=1) as persist,
        ):
            # a-side features, (13, NB, 128, 32): per batch, point p = 32r + j
            # lives at free position (r, j). Used as the MOVING side.
            lh6 = persist.tile([13, NB, P, JPP], bf16)
            a2pos = persist.tile([P, NB * JPP], f32)

            def emit_aside(b):
                # asr rows: 0-2 a_hi, 3-5 a_lo, 6-8 a_hi, 9-10 ones, 11-12
                # a2_hi/lo; DRAM bounce so the reload puts g on partitions
                ldq = nc.sync if b == 0 else nc.gpsimd
                as2 = prep.tile([P, JPP, 3], f32, tag="as2")
                ldq.dma_start(
                    as2[:],
                    skel.rearrange("(b r j) c -> b r (j c)", b=NB, r=P, j=JPP)[b],
                )
                sqa = prep.tile([P, JPP, 3], f32, tag="sqa")
                nc.scalar.square(sqa[:], as2[:])
                nc.vector.tensor_reduce(
                    a2pos[:, b * JPP : (b + 1) * JPP],
                    sqa[:],
                    axis=AX.X,
                    op=AL.add,
                )
                asr = prep.tile([P, 13, JPP], bf16, tag="asr")
                nc.vector.memset(asr[:], 1.0)
                ah_v = asr[:, 0:3, :].rearrange("r c j -> r j c")
                nc.vector.tensor_copy(ah_v, as2[:])
                nc.vector.tensor_copy(
                    asr[:, 6:9, :].rearrange("r c j -> r j c"), as2[:]
                )
                nc.vector.tensor_tensor(
                    out=asr[:, 3:6, :].rearrange("r c j -> r j c"),
                    in0=as2[:],
                    in1=ah_v,
                    op=AL.subtract,
                )
                a2s = a2pos[:, b * JPP : (b + 1) * JPP]
                nc.vector.tensor_copy(asr[:, 11, :], a2s)
                nc.vector.tensor_tensor(
                    out=asr[:, 12, :], in0=a2s, in1=asr[:, 11, :], op=AL.subtract
                )
                nc.gpsimd.dma_start(scratch_a[b], asr[:])
                dmae = nc.scalar if b == 0 else nc.gpsimd
                dmae.dma_start(
                    lh6[:, b],
                    scratch_a[b].rearrange("r (g j) -> g r j", g=13, j=JPP),
                )

            with tc.tile_pool(name="pprep", bufs=2, space="PSUM") as pprep:
                ident = constp.tile([P, P], f32)
                # ---------- B side: curve points -> bfeat (13, 2048) --------
                # Kick the b-side input DMAs FIRST: this chain (b6 -> cv ->
                # sb -> transpose -> bfeat scatter) is the prep critical path.
                # B6[3n+c, 16c+t] = 2*basis[t, n]  (block-diagonal over c)
                b6 = persist.tile([12, 48], f32)
                nc.vector.memset(b6[:], 0.0)
                _qs = [nc.sync, nc.scalar, nc.gpsimd]
                for c in range(3):
                    for n in range(4):
                        _qs[(3 * n + c) % 3].dma_start(
                            b6[3 * n + c : 3 * n + c + 1, 16 * c : 16 * c + 16],
                            basis[:, n : n + 1],
                        )
                pp = prep.tile([P, 12], f32)
                nc.sync.dma_start(pp[:], prim[:])
                nc.gpsimd.dma_start(ident[:], ident_dram[:])

                emit_aside(0)

                nc.scalar.mul(b6[:], b6[:], 2.0)
                ps_cpt = pprep.tile([12, P], f32)
                nc.tensor.transpose(ps_cpt[:], pp[:], ident[:])
                cpt = prep.tile([12, P], f32)
                nc.scalar.copy(cpt[:], ps_cpt[:])

                ps_cv = pprep.tile([P, 48], f32)
                nc.tensor.matmul(ps_cv[:], cpt[:], b6[:])  # (128,48) = 2*curves

                # sb bf16 (P,128): [0:48]=R0=bf16(2b), [48:96]=R1=2b-R0,
                # [96:112]=(-b^2)_hi, [112:128]=(-b^2)_lo
                sb = prep.tile([P, 128], bf16)
                nc.scalar.copy(sb[:, 0:48], ps_cv[:])
                nc.vector.tensor_tensor(
                    out=sb[:, 48:96], in0=ps_cv[:], in1=sb[:, 0:48], op=AL.subtract
                )
                sq = prep.tile([P, 48], f32)
                nc.scalar.activation(sq[:], ps_cv[:], ACT.Square, scale=0.5)
                nb2 = prep.tile([P, 16], f32)
                nc.vector.tensor_reduce(
                    nb2[:],
                    sq[:].rearrange("p (c t) -> p t c", c=3, t=16),
                    axis=AX.X,
                    op=AL.add,
                    negate=True,
                )
                nc.vector.tensor_copy(sb[:, 96:112], nb2[:])
                nc.vector.tensor_tensor(
                    out=sb[:, 112:128], in0=nb2[:], in1=sb[:, 96:112], op=AL.subtract
                )

                # Transpose sb on TensorE (no DRAM bounce): sbT[f, q] = sb[q, f].
                # bfeat columns use m = t*64 + prim ordering within each batch
                # (the colmin mean over m is order-invariant, so the host
                # mapping is unchanged).
                identb = constp.tile([P, P], bf16)
                nc.vector.tensor_copy(identb[:], ident[:])
                ps_t = pprep.tile([P, P], bf16)
                nc.tensor.transpose(ps_t[:], sb[:], identb[:])
                sbT = persist.tile([P, P], bf16)
                nc.vector.tensor_copy(sbT[:], ps_t[:])

                # bfeat rows: 0-2 R0, 3-5 R0, 6-8 R1, 9 (-b^2)hi, 10 (-b^2)lo,
                # 11-12 = -1. Used as the STATIONARY side (lhsT slices).
                bfeat = persist.tile([13, NB * M], bf16)
                nc.vector.memset(bfeat[:], -1.0)   # rows 11-12 stay -1

                _dq = [nc.sync, nc.scalar]
                groups = [(0, 3, 0, 48), (3, 9, 0, 96), (9, 11, 96, 128)]
                for i, (rlo, rhi, flo, fhi) in enumerate(groups):
                    for b in range(NB):
                        ov = bfeat[rlo:rhi, b * M : (b + 1) * M].rearrange(
                            "c (t p) -> c t p", t=16, p=64
                        )
                        _dq[(2 * i + b) % 2].dma_start(
                            ov, sbT[flo:fhi, b * 64 : (b + 1) * 64]
                        )

            # ---------------- main loop --------------------------------
            with (
                tc.tile_pool(name="mpsum", bufs=2, space="PSUM") as mpsum,
                tc.tile_pool(name="mout", bufs=1) as mout,
                tc.tile_pool(name="sbdp", bufs=5) as sbdp,
                tc.tile_pool(name="bodyp", bufs=3) as bodyp,
            ):
                colraw = mout.tile([P, NB * MB], f32)
                # ping-pong running-min buffers (rowmin side), per batch
                rp0a = mout.tile([P, PPB], bf16)
                rp1a = mout.tile([P, PPB], bf16)
                rp0b = mout.tile([P, PPB], bf16)
                rp1b = mout.tile([P, PPB], bf16)
                rps = [[rp0a, rp1a], [rp0b, rp1b]]

                def emit_main(b):
                    rp = rps[b]
                    deferred = []
                    aflat = lh6[:, b].rearrange("g r j -> g (r j)")  # (13, 4096)
                    for q in range(MB):
                        lhsTq = bfeat[:, b * M + q * P : b * M + (q + 1) * P]
                        if q == 0:
                            sbd = rp[0][:]
                        else:
                            sbdt = sbdp.tile([P, PPB], bf16, tag="sbd")
                            sbd = sbdt[:]
                        for h in range(2):
                            ps = mpsum.tile([P, HP], f32, tag="ps")
                            for i in range(4):
                                lo = h * HP + i * 512
                                nc.tensor.matmul(
                                    ps[:, i * 512 : (i + 1) * 512],
                                    lhsTq,
                                    aflat[:, lo : lo + 512],
                                )
                            # drain: sbd = Relu(-psum) = max(d2, 0) in bf16
                            nc.scalar.activation(
                                sbd[:, h * HP : (h + 1) * HP],
                                ps[:],
                                ACT.Copy,
                                bias=0.0,
                                scale=-1.0,
                            )
                        # TT first: it is on the critical chain to orun
                        if q > 0:
                            src = rp[(q - 1) % 2][:]
                            dst = rp[q % 2][:]
                            nc.vector.tensor_tensor(
                                out=dst, in0=sbd, in1=src, op=AL.min
                            )
                        def emit_custom(sbd=sbd, q=q):
                            body = bodyp.tile([P, HP], bf16, tag="body")
                            nc.vector._custom_dve(
                                min_op,
                                out=body[:],
                                in0=sbd[:, 0:HP],
                                in1=sbd[:, HP:PPB],
                                s0=3.0e38,
                                accum_out=colraw[
                                    :, b * MB + q : b * MB + q + 1
                                ],
                            )

                        if b == NB - 1 and q >= MB - 3:
                            # skip the DVE colmin custom for the last blocks:
                            # dump the tile and fold min_p on the host, so the
                            # DVE tail is only the final running-min TTs
                            nc.sync.dma_start(osbd[q - (MB - 3)], sbd)
                        else:
                            emit_custom()
                        if q == MB - 1:
                            fin = rp[(MB - 1) % 2]
                            nc.sync.dma_start(orun[b, :, 0:HP], fin[:, 0:HP])
                            nc.sync.dma_start(orun[b, :, HP:PPB], fin[:, HP:PPB])

                emit_main(0)
                emit_aside(1)
                emit_main(1)

                nc.sync.dma_start(ocolr[:], colraw[:])

    nc.compile()
    return nc


def _get_nc():
    if "nc" not in _CACHE:
        _CACHE["nc"] = _build_nc()
    return _CACHE["nc"]


def make_in_maps(skeletal_points, primitive_parameters, bspline_basis):
    skel = np.ascontiguousarray(skeletal_points, dtype=np.float32)
    prim = np.ascontiguousarray(primitive_parameters, dtype=np.float32)
    basis = np.ascontiguousarray(bspline_basis, dtype=np.float32)
    in_maps = []
    for c in range(NCORES):
        sk = skel[NB * c : NB * (c + 1)].reshape(NB * PPB, 3)
        pr = prim[NB * c : NB * (c + 1)].reshape(P, 12)
        in_maps.append(
            {
                "skel": np.ascontiguousarray(sk),
                "prim": np.ascontiguousarray(pr),
                "basis": basis,
            }
        )
    return in_maps


def _to_f32(a):
    a = np.asarray(a)
    if a.dtype == np.uint16 or a.dtype == np.int16:
        return (a.astype(np.uint32) << 16).view(np.float32).astype(np.float64)
    return a.astype(np.float64)


def postprocess(results):
    """results: list of 8 per-core dicts with orun/ocolr."""
    loss = np.zeros(16, dtype=np.float32)
    for c, r in enumerate(results):
        runs = _to_f32(r["orun"])      # (2, 128, 4096) d2, already relu'd
        colr = _to_f32(r["ocolr"])     # (128, 2*2*MB)
        osbd = _to_f32(r["osbd"])      # (3, 128, 4096) raw d2, b=1 q>=5
        for b in range(NB):
            rowmin = runs[b].min(axis=0)                       # (4096,)
            cha = np.sqrt(np.maximum(rowmin, 0.0)).mean()
            colmin = colr[:, b * MB : (b + 1) * MB].copy()     # (128, 8) m=q*128+r
            if b == NB - 1:
                colmin[:, MB - 3 :] = osbd.min(axis=2).T
            chb = np.sqrt(np.maximum(colmin, 0.0)).mean()
            loss[NB * c + b] = np.float32(cha + chb)
    return loss


def kernel(skeletal_points, primitive_parameters, bspline_basis):
    from concourse.bass_utils import run_bass_kernel_spmd

    nc = _get_nc()
    in_maps = make_in_maps(skeletal_points, primitive_parameters, bspline_basis)
    res = run_bass_kernel_spmd(nc, in_maps, core_ids=list(range(NCORES)))
    return postprocess(res.results)


# revision 47
# speedup vs baseline: 1.1907x; 1.1907x over previous
"""Trainium2 Bass kernel for nn_BsplineLoss (chamfer between skeletal points
and bspline curve points).

Full-input contract: kernel(**inputs) takes the unsharded arrays
  skeletal_points      (16, 4096, 3) f32
  primitive_parameters (16, 64, 12)  f32
  bspline_basis        (16, 4)       f32
and returns the full (16,) f32 loss.

Sharding: data-parallel over batch B=16 across 8 cores (2 batches/core).

Device algorithm (per core, per batch), "m-on-partitions" orientation:
  curves b = einsum(basis, ctrl)                       (M=1024 points)
  psum[m, p] = 2*a_p.b_m - |b_m|^2 - |a_p|^2 = -d2     (K=13 hi/lo bf16)
  for each m-block q (128 curve points on partitions, p free):
    sbd(q) = Relu(-psum) = d2  (bf16; two (128,2048) ScalarE drains)
    colraw[r, q] = min_p sbd   (one DVE custom 2-stream min+accum, FD=2048)
    run = min(run, sbd)        (one DVE tensor_tensor min, FD=4096, 2x mode)
Host: fold the 128-partition axis of run (rowmin over the 8 m-blocks is in
run already; partitions hold m mod 128) and relu/sqrt/mean -> loss; colmin
comes from colraw directly (m = q*128 + r).

The steady state is ScalarE-drain-paced (~2 x 1.97us per m-block); the DVE
(custom min at 1x + TT at 2x) runs ~0.5us/block behind, so the final batch's
last three colmin customs are skipped entirely: their raw d2 tiles are dumped
to DRAM on the idle sync HW-DGE queue and min-folded on the host (osbd).
"""

import numpy as np

P = 128
NB = 2           # batches per core
PPB = 4096       # skeletal points per batch
M = 1024         # curve points per batch
MB = 8           # m-blocks per batch (128 m's each)
HP = 2048        # p-half size
JPP = 32
NCORES = 8

_CACHE = {}


def _register_min_op():
    """Custom DVE op: out = min(in0, in1); accum_out = min(c0, min_k out).
    Reads two SBUF bf16 streams at 2 elem/cycle/lane total (2x perf mode)."""
    from concourse import dve_ops
    from concourse.dve_spec import Spec, minn, Src0, Src1, C0, lower, _has_src1
    from concourse.dve_uop import DveOpSpec

    name = "TT_MIN_RED_ANT"
    for o in dve_ops.OPS:
        if o.name == name:
            return o

    def _ref(in0, in1, c0, c1, c2):
        body = np.minimum(in0.astype(np.float32), in1.astype(np.float32))
        acc = np.minimum(
            c0, body.reshape(body.shape[0], -1).min(axis=-1, keepdims=True)
        )
        return body, acc

    spec = Spec(body=minn(Src0, Src1), accum=minn, accum_init=C0, reference=_ref)
    opcode = max(dve_ops._SUB_OPCODE_FOR_NAME.values()) + 1
    assert opcode < 0x20
    shas = {}
    for ver in ("v3", "v4"):
        try:
            s = DveOpSpec(
                name=name, opcode=opcode, uops=lower(spec, ver=ver),
                rd1_en=_has_src1(spec),
            )
            shas[ver] = s.sha(ver)
        except Exception:
            pass
    op = dve_ops.DveOp(name, spec, subdim=False, uops_sha=shas,
                       perf_en={"v3": True, "v4": True})
    dve_ops.OPS.append(op)
    dve_ops.CUSTOM_DVE_SPECS[name] = spec
    dve_ops._SUB_OPCODE_FOR_NAME[name] = opcode
    return op


def _build_nc():
    import concourse.bacc as bacc
    import concourse.bass as bass
    import concourse.tile as tile
    from concourse import mybir, bass_isa

    f32 = mybir.dt.float32
    bf16 = mybir.dt.bfloat16
    AX = mybir.AxisListType
    AL = mybir.AluOpType
    ACT = mybir.ActivationFunctionType

    min_op = _register_min_op()
    nc = bacc.Bacc(None, target_bir_lowering=False)

    skel = nc.dram_tensor("skel", [NB * PPB, 3], f32, kind="ExternalInput")
    prim = nc.dram_tensor("prim", [P, 12], f32, kind="ExternalInput")
    basis = nc.dram_tensor("basis", [16, 4], f32, kind="ExternalInput")

    # rowmin side: running min over m-blocks, host folds the 128 partitions
    orun = nc.dram_tensor("orun", [NB, P, PPB], bf16, kind="ExternalOutput")
    # colmin side: one free-axis min column per m-block (m = q*128 + r)
    ocolr = nc.dram_tensor("ocolr", [P, NB * MB], f32, kind="ExternalOutput")
    # raw d2 tiles for the final batch's last 3 m-blocks (host folds colmin);
    # dumped via the idle sync HW-DGE queue, NOT gpsimd SWDGE (too slow)
    osbd = nc.dram_tensor("osbd", [3, P, PPB], bf16, kind="ExternalOutput")

    scratch = nc.dram_tensor("scratch", [P, 128], bf16)
    scratch_a = nc.dram_tensor("scratch_a", [NB, P, 13 * JPP], bf16)

    ident_dram = nc.inline_tensor(np.eye(P, dtype=np.float32), name="ident")

    with tile.TileContext(nc) as tc:
        with (
            tc.tile_pool(name="const", bufs=1) as constp,
            tc.tile_pool(name="prep", bufs=2) as prep,
            tc.tile_pool(name="persist", bufs=1) as persist,
        ):
            # a-side features, (13, NB, 128, 32): per batch, point p = 32r + j
            # lives at free position (r, j). Used as the MOVING side.
            lh6 = persist.tile([13, NB, P, JPP], bf16)
            a2pos = persist.tile([P, NB * JPP], f32)

            def emit_aside(b):
                # asr rows: 0-2 a_hi, 3-5 a_lo, 6-8 a_hi, 9-10 ones, 11-12
                # a2_hi/lo; DRAM bounce so the reload puts g on partitions
                ldq = nc.sync if b == 0 else nc.gpsimd
                as2 = prep.tile([P, JPP, 3], f32, tag="as2")
                ldq.dma_start(
                    as2[:],
                    skel.rearrange("(b r j) c -> b r (j c)", b=NB, r=P, j=JPP)[b],
                )
                sqa = prep.tile([P, JPP, 3], f32, tag="sqa")
                nc.scalar.square(sqa[:], as2[:])
                nc.vector.tensor_reduce(
                    a2pos[:, b * JPP : (b + 1) * JPP],
                    sqa[:],
                    axis=AX.X,
                    op=AL.add,
                )
                asr = prep.tile([P, 13, JPP], bf16, tag="asr")
                nc.vector.memset(asr[:], 1.0)
                ah_v = asr[:, 0:3, :].rearrange("r c j -> r j c")
                nc.vector.tensor_copy(ah_v, as2[:])
                nc.vector.tensor_copy(
                    asr[:, 6:9, :].rearrange("r c j -> r j c"), as2[:]
                )
                nc.vector.tensor_tensor(
                    out=asr[:, 3:6, :].rearrange("r c j -> r j c"),
                    in0=as2[:],
                    in1=ah_v,
                    op=AL.subtract,
                )
                a2s = a2pos[:, b * JPP : (b + 1) * JPP]
                nc.vector.tensor_copy(asr[:, 11, :], a2s)
                nc.vector.tensor_tensor(
                    out=asr[:, 12, :], in0=a2s, in1=asr[:, 11, :], op=AL.subtract
                )
                nc.gpsimd.dma_start(scratch_a[b], asr[:])
                dmae = nc.scalar if b == 0 else nc.gpsimd
                dmae.dma_start(
                    lh6[:, b],
                    scratch_a[b].rearrange("r (g j) -> g r j", g=13, j=JPP),
                )

            with tc.tile_pool(name="pprep", bufs=2, space="PSUM") as pprep:
                ident = constp.tile([P, P], f32)
                # ---------- B side: curve points -> bfeat (13, 2048) --------
                # Kick the b-side input DMAs FIRST: this chain (b6 -> cv ->
                # sb -> transpose -> bfeat scatter) is the prep critical path.
                # B6[3n+c, 16c+t] = 2*basis[t, n]  (block-diagonal over c)
                b6 = persist.tile([12, 48], f32)
                nc.vector.memset(b6[:], 0.0)
                _qs = [nc.sync, nc.scalar, nc.gpsimd]
                for c in range(3):
                    for n in range(4):
                        _qs[(3 * n + c) % 3].dma_start(
                            b6[3 * n + c : 3 * n + c + 1, 16 * c : 16 * c + 16],
                            basis[:, n : n + 1],
                        )
                pp = prep.tile([P, 12], f32)
                nc.sync.dma_start(pp[:], prim[:])
                nc.gpsimd.dma_start(ident[:], ident_dram[:])

                emit_aside(0)

                nc.scalar.mul(b6[:], b6[:], 2.0)
                ps_cpt = pprep.tile([12, P], f32)
                nc.tensor.transpose(ps_cpt[:], pp[:], ident[:])
                cpt = prep.tile([12, P], f32)
                nc.scalar.copy(cpt[:], ps_cpt[:])

                ps_cv = pprep.tile([P, 48], f32)
                nc.tensor.matmul(ps_cv[:], cpt[:], b6[:])  # (128,48) = 2*curves

                # sb bf16 (P,128): [0:48]=R0=bf16(2b), [48:96]=R1=2b-R0,
                # [96:112]=(-b^2)_hi, [112:128]=(-b^2)_lo
                sb = prep.tile([P, 128], bf16)
                nc.scalar.copy(sb[:, 0:48], ps_cv[:])
                nc.vector.tensor_tensor(
                    out=sb[:, 48:96], in0=ps_cv[:], in1=sb[:, 0:48], op=AL.subtract
                )
                sq = prep.tile([P, 48], f32)
                nc.scalar.activation(sq[:], ps_cv[:], ACT.Square, scale=0.5)
                nb2 = prep.tile([P, 16], f32)
                nc.vector.tensor_reduce(
                    nb2[:],
                    sq[:].rearrange("p (c t) -> p t c", c=3, t=16),
                    axis=AX.X,
                    op=AL.add,
                    negate=True,
                )
                nc.vector.tensor_copy(sb[:, 96:112], nb2[:])
                nc.vector.tensor_tensor(
                    out=sb[:, 112:128], in0=nb2[:], in1=sb[:, 96:112], op=AL.subtract
                )

                # Transpose sb on TensorE (no DRAM bounce): sbT[f, q] = sb[q, f].
                # bfeat columns use m = t*64 + prim ordering within each batch
                # (the colmin mean over m is order-invariant, so the host
                # mapping is unchanged).
                identb = constp.tile([P, P], bf16)
                nc.vector.tensor_copy(identb[:], ident[:])
                ps_t = pprep.tile([P, P], bf16)
                nc.tensor.transpose(ps_t[:], sb[:], identb[:])
                sbT = persist.tile([P, P], bf16)
                nc.vector.tensor_copy(sbT[:], ps_t[:])

                # bfeat rows: 0-2 R0, 3-5 R0, 6-8 R1, 9 (-b^2)hi, 10 (-b^2)lo,
                # 11-12 = -1. Used as the STATIONARY side (lhsT slices).
                bfeat = persist.tile([13, NB * M], bf16)
                nc.vector.memset(bfeat[:], -1.0)   # rows 11-12 stay -1

                _dq = [nc.sync, nc.scalar]
                groups = [(0, 3, 0, 48), (3, 9, 0, 96), (9, 11, 96, 128)]
                for i, (rlo, rhi, flo, fhi) in enumerate(groups):
                    for b in range(NB):
                        ov = bfeat[rlo:rhi, b * M : (b + 1) * M].rearrange(
                            "c (t p) -> c t p", t=16, p=64
                        )
                        _dq[(2 * i + b) % 2].dma_start(
                            ov, sbT[flo:fhi, b * 64 : (b + 1) * 64]
                        )

            # ---------------- main loop --------------------------------
            with (
                tc.tile_pool(name="mpsum", bufs=2, space="PSUM") as mpsum,
                tc.tile_pool(name="mout", bufs=1) as mout,
                tc.tile_pool(name="sbdp", bufs=4) as sbdp,
                tc.tile_pool(name="bodyp", bufs=3) as bodyp,
            ):
                colraw = mout.tile([P, NB * MB], f32)
                # ping-pong running-min buffers (rowmin side), per batch
                rp0a = mout.tile([P, PPB], bf16)
                rp1a = mout.tile([P, PPB], bf16)
                rp0b = mout.tile([P, PPB], bf16)
                rp1b = mout.tile([P, PPB], bf16)
                rps = [[rp0a, rp1a], [rp0b, rp1b]]

                def emit_main(b):
                    rp = rps[b]
                    deferred = []
                    aflat = lh6[:, b].rearrange("g r j -> g (r j)")  # (13, 4096)
                    for q in range(MB):
                        lhsTq = bfeat[:, b * M + q * P : b * M + (q + 1) * P]
                        if q == 0:
                            sbd = rp[0][:]
                        else:
                            sbdt = sbdp.tile([P, PPB], bf16, tag="sbd")
                            sbd = sbdt[:]
                        for h in range(2):
                            ps = mpsum.tile([P, HP], f32, tag="ps")
                            for i in range(4):
                                lo = h * HP + i * 512
                                nc.tensor.matmul(
                                    ps[:, i * 512 : (i + 1) * 512],
                                    lhsTq,
                                    aflat[:, lo : lo + 512],
                                )
                            # drain: sbd = Relu(-psum) = max(d2, 0) in bf16
                            nc.scalar.activation(
                                sbd[:, h * HP : (h + 1) * HP],
                                ps[:],
                                ACT.Copy,
                                bias=0.0,
                                scale=-1.0,
                            )
                        # TT first: it is on the critical chain to orun
                        if q > 0:
                            src = rp[(q - 1) % 2][:]
                            dst = rp[q % 2][:]
                            nc.vector.tensor_tensor(
                                out=dst, in0=sbd, in1=src, op=AL.min
                            )
                        def emit_custom(sbd=sbd, q=q):
                            body = bodyp.tile([P, HP], bf16, tag="body")
                            nc.vector._custom_dve(
                                min_op,
                                out=body[:],
                                in0=sbd[:, 0:HP],
                                in1=sbd[:, HP:PPB],
                                s0=3.0e38,
                                accum_out=colraw[
                                    :, b * MB + q : b * MB + q + 1
                                ],
                            )

                        if b == NB - 1 and q >= MB - 3:
                            # skip the DVE colmin custom for the last blocks:
                            # dump the tile and fold min_p on the host, so the
                            # DVE tail is only the final running-min TTs
                            nc.sync.dma_start(osbd[q - (MB - 3)], sbd)
                        else:
                            emit_custom()
                        if q == MB - 1:
                            fin = rp[(MB - 1) % 2]
                            nc.sync.dma_start(orun[b, :, 0:HP], fin[:, 0:HP])
                            nc.sync.dma_start(orun[b, :, HP:PPB], fin[:, HP:PPB])

                emit_main(0)
                emit_aside(1)
                emit_main(1)

                nc.sync.dma_start(ocolr[:], colraw[:])

    nc.compile()
    return nc


def _get_nc():
    if "nc" not in _CACHE:
        _CACHE["nc"] = _build_nc()
    return _CACHE["nc"]


def make_in_maps(skeletal_points, primitive_parameters, bspline_basis):
    skel = np.ascontiguousarray(skeletal_points, dtype=np.float32)
    prim = np.ascontiguousarray(primitive_parameters, dtype=np.float32)
    basis = np.ascontiguousarray(bspline_basis, dtype=np.float32)
    in_maps = []
    for c in range(NCORES):
        sk = skel[NB * c : NB * (c + 1)].reshape(NB * PPB, 3)
        pr = prim[NB * c : NB * (c + 1)].reshape(P, 12)
        in_maps.append(
            {
                "skel": np.ascontiguousarray(sk),
                "prim": np.ascontiguousarray(pr),
                "basis": basis,
            }
        )
    return in_maps


def _to_f32(a):
    a = np.asarray(a)
    if a.dtype == np.uint16 or a.dtype == np.int16:
        return (a.astype(np.uint32) << 16).view(np.float32).astype(np.float64)
    return a.astype(np.float64)


def postprocess(results):
    """results: list of 8 per-core dicts with orun/ocolr."""
    loss = np.zeros(16, dtype=np.float32)
    for c, r in enumerate(results):
        runs = _to_f32(r["orun"])      # (2, 128, 4096) d2, already relu'd
        colr = _to_f32(r["ocolr"])     # (128, 2*2*MB)
        osbd = _to_f32(r["osbd"])      # (3, 128, 4096) raw d2, b=1 q>=5
        for b in range(NB):
            rowmin = runs[b].min(axis=0)                       # (4096,)
            cha = np.sqrt(np.maximum(rowmin, 0.0)).mean()
            colmin = colr[:, b * MB : (b + 1) * MB].copy()     # (128, 8) m=q*128+r
            if b == NB - 1:
                colmin[:, MB - 3 :] = osbd.min(axis=2).T
            chb = np.sqrt(np.maximum(colmin, 0.0)).mean()
            loss[NB * c + b] = np.float32(cha + chb)
    return loss


def kernel(skeletal_points, primitive_parameters, bspline_basis):
    from concourse.bass_utils import run_bass_kernel_spmd

    nc = _get_nc()
    in_maps = make_in_maps(skeletal_points, primitive_parameters, bspline_basis)
    res = run_bass_kernel_spmd(nc, in_maps, core_ids=list(range(NCORES)))
    return postprocess(res.results)
